# revision 6
# baseline (speedup 1.0000x reference)
"""Trainium2 Bass kernel for nn_BuildVolume (deformable multi-view bilinear
sampling + gating + 1x1 conv), SPMD over 8 NeuronCores.

Algorithm (exact, no gathers):
  bilinear sample = sum over grid of hat(ix-x)*hat(iy-y)*img[y,x,c]
  (hat(u) = relu(1-|u|); out-of-image grid points contribute 0, matching
  the reference's padding_mode='zeros').

  Per view v=(vv,uu): stage 1 contracts the x axis on the TensorEngine
  (lhsT = per-sample x-hat matrix, shared across the vv's of a uu),
  payload = (c, y-window); stage 2 applies the y-hat and reduces on the
  Vector engine; the gate multiplies in; the 1x1 conv is a matmul over
  (c,v) with the cost tile DMA-transposed.

Sharding: output rows H are split 8 x 16; each core reads a padded row
band of the images. No collectives.
"""

import numpy as np
import ml_dtypes

import concourse.bacc as bacc
import concourse.mybir as mybir
from concourse.tile import TileContext
from concourse import bass_utils

F32 = mybir.dt.float32
BF16 = mybir.dt.bfloat16
AX = mybir.AxisListType
OP = mybir.AluOpType
ACTF = mybir.ActivationFunctionType

# problem dims
B, H, W, N, C, OUT, M = 1, 128, 128, 4, 8, 64, 9
V = M * M
NCORE = 8
PAD = 21                    # spatial halo; covers |deltmap| <= 4.8
YW = 43                     # y window rows per output row
NWB = 4                     # w blocks of 32 (x128 samples each)
SB = 128                    # samples per block = 32 w * 4 n
NCH = 3                     # v chunks (3 rows of vv each -> 27 views)
VCH = 27
CVL = VCH * C               # 216 -> padded 256 local conv channels
XP = 74                     # slab partitions (window width per w-block)

HPC_FULL = H // NCORE       # 16


def _bf16(a):
    return np.asarray(a, dtype=ml_dtypes.bfloat16)


def prep_core(k, deltmap, imageMxM, x_g, conv_w, conv_b, hpc=HPC_FULL):
    """Host-side shard prep for core k. Pure numpy layout work."""
    yc = hpc - 1 + YW
    d = deltmap[0]                       # [H, W, N]
    hg0 = hpc * k
    sfull = np.arange(NWB * SB)
    wg = sfull // 4                      # 0..127
    nn_ = sfull % 4
    aw_pad = wg * (W / (W - 1.0)) - 0.5 + PAD

    # img: [NCH, XP, VCH*NWB*yc*C] bf16, zero padded, per-wb x windows
    ypad = np.zeros((V, yc, W + 2 * PAD, C), np.float32)
    y0g = hg0 - PAD
    ys = max(0, y0g)
    ye = min(H, y0g + yc)
    pl = np.moveaxis(imageMxM[0], (3, 4), (0, 1)).reshape(V, H, W, C)
    ypad[:, ys - y0g:ye - y0g, PAD:PAD + W, :] = pl[:, ys:ye]
    img = np.zeros((NCH, XP, VCH, NWB, yc * C), np.float32)
    for wb in range(NWB):
        xs = wb * 32
        blk = ypad[:, :, xs:xs + XP, :]            # [V, yc, XP, C]
        blk = np.transpose(blk, (2, 0, 1, 3))      # [XP, V, yc, C]
        for ch in range(NCH):
            img[ch, :, :, wb, :] = blk[:, ch * VCH:(ch + 1) * VCH].reshape(
                XP, VCH, yc * C)
    img = _bf16(img.reshape(NCH, XP, VCH * NWB * yc * C))

    # tb: [hpc, XP, 512] f32  (t broadcast along partitions)
    trow = d[hg0:hg0 + hpc].reshape(hpc, W * N)    # (w,n) == sfull order
    tb = np.broadcast_to(trow[:, None, :], (hpc, XP, NWB * SB)).copy()

    # dcol: [128 s, hpc*NWB] f32
    dcol = np.zeros((SB, hpc * NWB), np.float32)
    s128 = np.arange(SB)
    for h in range(hpc):
        for wb in range(NWB):
            dcol[:, h * NWB + wb] = d[hg0 + h, wb * 32 + s128 // 4, s128 % 4]

    # awx consts: [XP, 4*SB]; col sfull carries its wb offset
    awx = np.zeros((XP, 4 * SB), np.float32)
    p = np.arange(XP)
    awx[:, :] = aw_pad[None, :] - (sfull // SB * 32)[None, :] - p[:, None]

    # yio2[s, h, d] = d - (ah_local - h): folds the per-h y offset
    yio = np.zeros((SB, hpc * YW), np.float32)
    for h in range(hpc):
        chh = (hg0 + h) * (H / (H - 1.0)) - 0.5 - hg0 + PAD - h
        yio[:, h * YW:(h + 1) * YW] = np.arange(YW, dtype=np.float32) - chh

    # xg: [hpc, 128, NWB*V] bf16 (wb-major, v-minor)
    g = x_g[0, 0]                        # [H, W, V]
    xg = np.zeros((hpc, SB, NWB * V), np.float32)
    for h in range(hpc):
        for wb in range(NWB):
            xg[h, :, wb * V:(wb + 1) * V] = g[hg0 + h, wb * 32 + s128 // 4, :]
    xg = _bf16(xg)

    # wt: [128, NCH*2*OUT] bf16; row p, (chunk, kt, o)
    wt = np.zeros((SB, NCH * 2 * OUT), np.float32)
    for ch in range(NCH):
        for kt in range(2):
            for pp in range(SB):
                kl = kt * SB + pp
                if kl < CVL:
                    vloc = kl // C
                    cc = kl % C
                    vg = ch * VCH + vloc
                    wt[pp, (ch * 2 + kt) * OUT:(ch * 2 + kt + 1) * OUT] = \
                        conv_w[:, cc * V + vg]
    wt = _bf16(wt)

    bias = np.tile(conv_b.reshape(OUT, 1), (2, 1)).astype(np.float32)

    return dict(img=img.reshape(NCH * XP, VCH * NWB * yc * C),
                tb=tb, dcol=dcol, awx=awx, yio=yio, xg=xg, wt=wt,
                bias=bias)


def build_nc(hpc=HPC_FULL):
    yc = hpc - 1 + YW
    nc = bacc.Bacc("TRN2", target_bir_lowering=False)

    img_d = nc.dram_tensor("img", [NCH * XP, VCH * NWB * yc * C], BF16,
                           kind="ExternalInput")
    tb_d = nc.dram_tensor("tb", [hpc, XP, NWB * SB], F32, kind="ExternalInput")
    dcol_d = nc.dram_tensor("dcol", [SB, hpc * NWB], F32, kind="ExternalInput")
    awx_d = nc.dram_tensor("awx", [XP, 4 * SB], F32, kind="ExternalInput")
    yio_d = nc.dram_tensor("yio", [SB, hpc * YW], F32,
                           kind="ExternalInput")
    xg_d = nc.dram_tensor("xg", [hpc, SB, NWB * V], BF16, kind="ExternalInput")
    wt_d = nc.dram_tensor("wt", [SB, NCH * 2 * OUT], BF16,
                          kind="ExternalInput")
    bias_d = nc.dram_tensor("bias", [2 * OUT, 1], F32, kind="ExternalInput")
    out_d = nc.dram_tensor("out", [OUT, hpc, W, N], F32, kind="ExternalOutput")

    with TileContext(nc) as tc:
        with (
            tc.tile_pool(name="consts", bufs=1) as cp,
            tc.tile_pool(name="slab", bufs=1) as sp,
            tc.tile_pool(name="hats", bufs=1) as hp,
            tc.tile_pool(name="work", bufs=2) as wp,
            tc.tile_pool(name="one", bufs=1) as op1,
            tc.tile_pool(name="scr", bufs=3) as scr,
            tc.tile_pool(name="pt", bufs=2, space="PSUM") as pt,
            tc.tile_pool(name="pc", bufs=2, space="PSUM") as pc,
        ):
            dcolT = cp.tile([SB, hpc * NWB], F32)
            nc.sync.dma_start(dcolT[:], dcol_d[:])
            awxT = cp.tile([XP, 4 * SB], F32)
            nc.sync.dma_start(awxT[:], awx_d[:])
            yioT = cp.tile([SB, hpc * YW], F32)
            nc.sync.dma_start(yioT[:], yio_d[:])
            wtT = cp.tile([SB, NCH * 2 * OUT], BF16)
            nc.sync.dma_start(wtT[:], wt_d[:])
            biasT = cp.tile([2 * OUT, 1], F32)
            nc.sync.dma_start(biasT[:], bias_d[:])

            partials = cp.tile([SB, hpc * NWB // 2 * SB], F32)

            img3 = img_d[:].rearrange("(ch p) f -> ch p f", ch=NCH, p=XP)

            for ch in range(NCH):
                slab = sp.tile([XP, VCH * NWB * yc * C], BF16, name="slab",
                               tag="slab")
                nc.sync.dma_start(slab[:], img3[ch])
                s4 = slab[:].rearrange("p (v wb y c) -> p v wb y c",
                                       v=VCH, wb=NWB, y=yc)

                for h in range(hpc):
                    tbT = op1.tile([XP, NWB * SB], F32, name="tbT", tag="tb")
                    nc.sync.dma_start(tbT[:], tb_d[h])
                    xgT = op1.tile([SB, NWB * V], BF16, name="xgT", tag="xg")
                    nc.sync.dma_start(xgT[:], xg_d[h])

                    # ---- y hats for this chunk's 3 vv rows: [128,(3,4,43)]
                    hyT = hp.tile([SB, 3 * NWB * YW], F32, name="hyT",
                                  tag="hy")
                    hy4 = hyT[:].rearrange("p (j wb y) -> p j wb y",
                                           j=3, wb=NWB)
                    for jl in range(3):
                        vv = ch * 3 + jl
                        jc = float(M // 2 - vv)
                        m0 = scr.tile([SB, NWB], F32, name="m0", tag="scr")
                        nc.vector.tensor_scalar_mul(
                            m0[:], dcolT[:, h * NWB:(h + 1) * NWB], jc)
                        for wb in range(NWB):
                            nc.vector.tensor_scalar(
                                hy4[:, jl, wb, :],
                                yioT[:, h * YW:(h + 1) * YW],
                                m0[:, wb:wb + 1], None, OP.subtract)
                        nc.scalar.activation(
                            hy4[:, jl], hy4[:, jl], ACTF.Abs)
                        nc.scalar.activation(
                            hy4[:, jl], hy4[:, jl], ACTF.Relu,
                            bias=1.0, scale=-1.0)

                    # ---- x hats: [74, 9*4*128] bf16, absolute offsets
                    hxT = hp.tile([XP, M * NWB * SB], BF16, name="hxT",
                                  tag="hx")
                    for ui in range(M):
                        ic = float(M // 2 - ui)
                        u = scr.tile([XP, NWB * SB], F32, name="u",
                                     tag="scr")
                        nc.vector.tensor_scalar_mul(u[:], tbT[:], ic)
                        nc.vector.tensor_tensor(u[:], u[:], awxT[:], OP.add)
                        nc.scalar.activation(u[:], u[:], ACTF.Abs)
                        nc.scalar.activation(
                            hxT[:, ui * NWB * SB:(ui + 1) * NWB * SB], u[:],
                            ACTF.Relu, bias=1.0, scale=-1.0)

                    # ---- cost tile for this (chunk, h)
                    cost = wp.tile([SB, NWB * 2 * SB], BF16, name="cost",
                                   tag="cost")
                    c4 = cost[:].rearrange("p (wb k) -> p wb k", wb=NWB)
                    nc.vector.memset(c4[:, :, CVL:], 0.0)

                    for wbp in range(2):
                        for ui in range(M):
                            for jl in range(3):
                                vloc = jl * M + ui
                                vg = (ch * 3 + jl) * M + ui
                                Tps = pt.tile([SB, 2 * 512], F32, name="Tps",
                                              tag="Tps")
                                for wi in range(2):
                                    wb = wbp * 2 + wi
                                    rhs = s4[:, vloc, wb, h:h + YW, :] \
                                        .rearrange("p y c -> p c y")
                                    lhsT = hxT[:,
                                               ui * NWB * SB + wb * SB:
                                               ui * NWB * SB + (wb + 1) * SB]
                                    nc.tensor.matmul(
                                        Tps[:, wi * 512:wi * 512 + C * YW],
                                        lhsT, rhs, start=True, stop=True)
                                # stage 2: y-hat multiply + reduce + gate
                                mlt = scr.tile([SB, 2 * C * YW], BF16,
                                               name="mlt", tag="scr")
                                m4 = mlt[:].rearrange(
                                    "p (wi c y) -> p wi c y", wi=2, c=C)
                                t4 = Tps[:].rearrange(
                                    "p (wi x) -> p wi x", wi=2)[:, :, :C * YW]
                                t4 = t4.rearrange("p wi (c y) -> p wi c y",
                                                  c=C)
                                hyv = hy4[:, jl, wbp * 2:(wbp + 1) * 2, :] \
                                    .unsqueeze(2).broadcast_to((SB, 2, C, YW))
                                nc.vector.tensor_tensor(m4[:], t4, hyv,
                                                        OP.mult)
                                red = scr.tile([SB, 2 * C], F32, name="red",
                                               tag="scr2", bufs=3)
                                r3 = red[:].rearrange("p (wi c) -> p wi c",
                                                      wi=2)
                                nc.vector.tensor_reduce(
                                    r3, m4[:], AX.X, OP.add)
                                gv = xgT[:].rearrange(
                                    "p (wb v) -> p wb v", wb=NWB)[
                                    :, wbp * 2:(wbp + 1) * 2, vg:vg + 1] \
                                    .broadcast_to((SB, 2, C))
                                nc.vector.tensor_tensor(
                                    c4[:, wbp * 2:(wbp + 1) * 2,
                                       vloc * C:(vloc + 1) * C],
                                    r3, gv, OP.mult)

                    # ---- conv for this (chunk, h): accumulate into partials
                    convp = None
                    for wb in range(NWB):
                        costT = wp.tile([SB, 2 * SB], BF16, name="costT",
                                        tag="costT")
                        for kt in range(2):
                            nc.sync.dma_start(
                                costT[:, kt * SB:(kt + 1) * SB],
                                c4[:, wb, kt * SB:(kt + 1) * SB],
                                transpose=True)
                        half = wb % 2
                        slot = (h * NWB + wb) // 2
                        if half == 0:
                            convp = pc.tile([SB, SB], F32, name="convp",
                                            tag="convp")
                        for kt in range(2):
                            nc.tensor.matmul(
                                convp[half * OUT:(half + 1) * OUT, :],
                                wtT[:, (ch * 2 + kt) * OUT:
                                    (ch * 2 + kt + 1) * OUT],
                                costT[:, kt * SB:(kt + 1) * SB],
                                start=(kt == 0), stop=(kt == 1))
                        if half == 1:
                            ps = partials[:, slot * SB:(slot + 1) * SB]
                            if ch == 0:
                                nc.vector.tensor_copy(ps, convp[:])
                            else:
                                nc.vector.tensor_tensor(ps, ps, convp[:],
                                                        OP.add)

            # ---- epilogue: bias + output DMA
            ob = out_d[:].rearrange("o hh w n -> o hh (w n)")
            for h in range(hpc):
                for wb in range(NWB):
                    half = wb % 2
                    slot = (h * NWB + wb) // 2
                    osb = wp.tile([OUT, SB], F32, name="osb", tag="osb")
                    nc.scalar.add(
                        osb[:],
                        partials[half * OUT:(half + 1) * OUT,
                                 slot * SB:(slot + 1) * SB],
                        biasT[half * OUT:(half + 1) * OUT, 0:1])
                    nc.sync.dma_start(
                        ob[:, h, wb * 32 * N:(wb + 1) * 32 * N], osb[:])

    nc.compile()
    return nc


_CACHE = {}


def kernel(deltmap, imageMxM, x_g, conv_w, conv_b):
    deltmap = np.asarray(deltmap)
    imageMxM = np.asarray(imageMxM)
    x_g = np.asarray(x_g)
    conv_w = np.asarray(conv_w)
    conv_b = np.asarray(conv_b)
    assert np.abs(deltmap).max() * 4.0 + 1.5 < PAD + 1, "halo too small"

    in_maps = [prep_core(k, deltmap, imageMxM, x_g, conv_w, conv_b)
               for k in range(NCORE)]

    if "nc" not in _CACHE:
        _CACHE["nc"] = build_nc()
    nc = _CACHE["nc"]

    res = bass_utils.run_bass_kernel_spmd(
        nc, in_maps, core_ids=list(range(NCORE)))
    outs = [res.results[k]["out"] for k in range(NCORE)]
    full = np.concatenate(outs, axis=1)                   # [64,128,128,4]
    return full[None].astype(np.float32)
